# revision 17
# baseline (speedup 1.0000x reference)
"""CLCE loss kernel for Trainium2 (8 NeuronCores, SPMD).

Loss = 0.5 * cl + 0.5 * ce where
  cl_i = logsumexp(loss_temp_i) - slot0_i   over a [N, 2N-1] packed row
  ce   = cross-entropy of y_pred vs y_true.

Decomposition (exact, validated in f64 against the reference formula):
  cl_i = log(exp(slot0_i) + (T_i - P_i) + (2N-2 - num_neg_i)) - slot0_i
where
  T_i  = sum_j exp((xn_i . xn_j + 1) * 0.25)      <- the O(N^2 D) part, on device
  P_i  = sum_{j: y_j = y_i} exp(sim_ij)           <- O(N * class_size), on host
  slot0_i = sim_{i, first same-class j != i}      <- O(N), on host
  R_i  = sum_j exp(y_pred_ij)                     <- on device
  ce_i = log(R_i) - y_pred[i, y_i]

Device sharding: core c computes rows [512c, 512(c+1)) of the similarity
matrix: a [512, 4096] x [4096(k=1024 contraction)] block matmul in fp32r
(full-rate tf32-precision PE path), fused exp+row-sum on the Scalar engine
(activation accum_out), and the CE row-sums for its block.
"""

import os
from contextlib import ExitStack

import numpy as np

import concourse.bass as bass
import concourse.tile as tile
from concourse import bacc, mybir
from concourse.bass_utils import run_bass_kernel_spmd

N, D, C = 4096, 1024, 512
TAU = 0.5
LAMBD = 0.5
NCORES = 8
BLK = N // NCORES          # 512 rows per core
P = 128                    # partitions
KT = D // P                # 8 contraction tiles
MT = BLK // P              # 4 output row tiles per core
HT = N // 512              # 8 column chunks of 512

_F32 = mybir.dt.float32
_F32R = mybir.dt.float32r
_EXP = mybir.ActivationFunctionType.Exp


def _build_kernel(tc, xt, wt, yp, out):
    """Emit the per-core Tile kernel.

    xt:  [D, N]   f32  normalized embeddings, transposed (d on rows) - shared
    wt:  [D, BLK] f32  this core's column block of xt (the stationary operand)
    yp:  [P, MT*C] f32 this core's y_pred block, partition-major packed
    out: [P, 2*MT] f32 col m: T row-sums, col MT+m: R row-sums
    """
    nc = tc.nc
    W = 1024                   # column-chunk width (2 psum banks)
    HC = N // W                # 4 chunks
    NS = W // 512              # matmuls per chunk k-step
    with ExitStack() as ctx:
        pers = ctx.enter_context(tc.tile_pool(name="pers", bufs=1))
        epool = ctx.enter_context(tc.tile_pool(name="epool", bufs=2))
        psum = ctx.enter_context(
            tc.tile_pool(name="psum", bufs=4, space=bass.MemorySpace.PSUM)
        )

        # per-(k, h) input tiles -> exact DMA->matmul dependencies
        XT = [
            [
                pers.tile([P, W], _F32R, name=f"xtt{k}_{h}", tag=f"xtt{k}_{h}")
                for h in range(HC)
            ]
            for k in range(KT)
        ]
        WT = [
            pers.tile([P, BLK], _F32R, name=f"wtt{k}", tag=f"wtt{k}")
            for k in range(KT)
        ]
        YPB = pers.tile([P, MT * C], _F32)     # 8 KiB/partition
        # out layout: [Tparts (MT*HC) | Rparts (MT)]
        OUTSB = pers.tile([P, MT * HC + MT], _F32)
        bias_s = pers.tile([P, 1], _F32)       # 0.5*TAU for the sim affine
        bias_z = pers.tile([P, 1], _F32)       # 0.0 for plain exp
        warm = pers.tile([P, 1], _F32)

        ZW = pers.tile([P, 512], mybir.dt.bfloat16)  # zeros, PE warm-up operand

        nc.gpsimd.memset(ZW[:], 0.0)
        nc.gpsimd.memset(bias_s[:], 0.5 * TAU)
        nc.gpsimd.memset(bias_z[:], 0.0)
        # warm the exp table (ACT_TABLE_LOAD ~2.7us) before any data lands
        nc.scalar.activation(warm[:], bias_z[:], _EXP, bias=bias_z[:], scale=1.0)

        # PE warm-up: ~10 dummy matmuls (~4us busy) during the input-DMA
        # ramp flip the HAM clock gate to 8/8 before the real stream starts
        wps = psum.tile([P, W], _F32, tag="ps")
        for _ in range(10):
            nc.tensor.matmul(wps[:, 0:512], ZW[:, 0:P], ZW[:], start=True, stop=True)

        # --- input DMAs.  Sync HWDGE carries the matmul operands in exactly
        # the order the PE consumes them: (WT k, XT[k][0]) pairs pace the
        # first chunk, then the later column chunks.  y_pred rides the
        # scalar HWDGE queue so it neither delays the sync stream nor the
        # CE activations. ---
        nc.scalar.dma_start(YPB[:], yp[:])
        for k in range(KT):
            nc.sync.dma_start(WT[k][:], wt[k * P:(k + 1) * P, :])
            nc.sync.dma_start(XT[k][0][:], xt[k * P:(k + 1) * P, 0:W])
        for h in range(1, HC):
            for k in range(KT):
                nc.sync.dma_start(
                    XT[k][h][:],
                    xt[k * P:(k + 1) * P, h * W:(h + 1) * W],
                )

        # --- CE: R[p, t] = sum_c exp(y_pred) ---
        for t in range(MT):
            et = epool.tile([P, W], _F32)
            nc.scalar.activation(
                et[:, 0:C], YPB[:, t * C:(t + 1) * C], _EXP,
                bias=bias_z[:], scale=1.0,
                accum_out=OUTSB[:, MT * HC + t:MT * HC + t + 1],
            )

        # --- main: sim block matmul + fused exp/row-sum ---
        # sim = (dot + 1) * 0.5 * TAU  ->  exp(scale*dot + bias) with
        # scale = bias = 0.5*TAU = 0.25
        for h in range(HC):
            for m in range(MT):
                ps = psum.tile([P, W], _F32)
                for k in range(KT):
                    for ns in range(NS):
                        nc.tensor.matmul(
                            ps[:, ns * 512:(ns + 1) * 512],
                            WT[k][:, m * P:(m + 1) * P],
                            XT[k][h][:, ns * 512:(ns + 1) * 512],
                            start=(k == 0),
                            stop=(k == KT - 1),
                        )
                et = epool.tile([P, W], _F32)
                nc.scalar.activation(
                    et[:], ps[:], _EXP,
                    bias=bias_s[:], scale=0.5 * TAU,
                    accum_out=OUTSB[:, m * HC + h:m * HC + h + 1],
                )

        nc.scalar.dma_start(out[:], OUTSB[:])


_NC_CACHE = None


def _get_nc():
    global _NC_CACHE
    if _NC_CACHE is None:
        nc = bacc.Bacc(
            "TRN2", target_bir_lowering=False, debug=False,
            enable_asserts=False, num_devices=NCORES,
        )
        xt_d = nc.dram_tensor("xt", [D, N], _F32R, kind="ExternalInput")
        wt_d = nc.dram_tensor("wt", [D, BLK], _F32R, kind="ExternalInput")
        yp_d = nc.dram_tensor("yp", [P, MT * C], _F32, kind="ExternalInput")
        HC = N // 1024
        out_d = nc.dram_tensor(
            "out", [P, MT * HC + MT], _F32, kind="ExternalOutput"
        )
        with tile.TileContext(nc) as tc:
            _build_kernel(tc, xt_d.ap(), wt_d.ap(), yp_d.ap(), out_d.ap())
        nc.compile()
        _NC_CACHE = nc
    return _NC_CACHE


def _run_device(xnT, y_pred, trace=False):
    """Run the SPMD kernel; returns (T[N], R[N]) f32 and the raw results."""
    in_maps = []
    for c in range(NCORES):
        blk = slice(c * BLK, (c + 1) * BLK)
        ypb = (
            np.ascontiguousarray(y_pred[blk])
            .reshape(MT, P, C).transpose(1, 0, 2).reshape(P, MT * C)
        )
        in_maps.append({
            "xt": xnT,
            "wt": np.ascontiguousarray(xnT[:, blk]),
            "yp": np.ascontiguousarray(ypb),
        })
    res = run_bass_kernel_spmd(
        _get_nc(), in_maps, core_ids=list(range(NCORES)), trace=trace,
    )
    HC = N // 1024
    T = np.empty(N, np.float64)
    R = np.empty(N, np.float64)
    for c, r in enumerate(res.results):
        o = r["out"].astype(np.float64)  # [128, MT*HC + MT]
        for m in range(MT):
            rows = slice(c * BLK + m * P, c * BLK + (m + 1) * P)
            T[rows] = o[:, m * HC:(m + 1) * HC].sum(axis=1)
            R[rows] = o[:, MT * HC + m]
    return T, R, res


def kernel(layer_embeds, y_true, y_pred):
    x = np.asarray(layer_embeds, dtype=np.float32)
    yt = np.asarray(y_true).astype(np.int64)
    yp = np.asarray(y_pred, dtype=np.float32)

    # normalize rows (torch-style eps clip)
    norms = np.maximum(
        np.sqrt((x.astype(np.float64) ** 2).sum(1, keepdims=True)), 1e-8
    )
    xn = (x / norms).astype(np.float32)
    xnT = np.ascontiguousarray(xn.T)  # [D, N]

    trace = bool(int(os.environ.get("CLCE_TRACE", "0")))
    T, R, res = _run_device(xnT, yp, trace=trace)
    if trace:
        kernel.last_results = res

    # --- host-side small terms (O(N * class_size)) ---
    counts = np.bincount(yt, minlength=C)
    P_ = np.zeros(N, np.float64)
    slot0 = np.zeros(N, np.float64)
    for cval in np.unique(yt):
        idx = np.where(yt == cval)[0]
        sub = xn[idx]
        s = ((sub @ sub.T).astype(np.float64) + 1.0) * (0.5 * TAU)
        P_[idx] = np.exp(s).sum(1)
        if len(idx) >= 2:
            firstpos = np.where(np.arange(len(idx)) == 0, 1, 0)
            slot0[idx] = s[np.arange(len(idx)), firstpos]

    num_neg = N - counts[yt]
    S = T.astype(np.float64) - P_
    Z = (2 * N - 2 - num_neg).astype(np.float64)
    cl = (np.log(np.exp(slot0) + S + Z) - slot0).mean()
    ce = (
        np.log(R.astype(np.float64)) - yp[np.arange(N), yt].astype(np.float64)
    ).mean()
    loss = LAMBD * cl + (1.0 - LAMBD) * ce
    return np.asarray(loss, dtype=np.float32)
